# revision 1
# baseline (speedup 1.0000x reference)
"""CenterLoss kernel for 8 Trainium2 NeuronCores.

loss = mean(distmat * onehot(labels)) over a (B, C) distmat where
distmat[i, j] = ||x_i - c_j||^2.  The mask selects exactly one element
per row, so  loss = (1/(B*C)) * sum_i ||x_i - c_{labels[i]}||^2.

Strategy: data-parallel over batch.  Each of the 8 cores takes 512 rows
of x, gathers its 512 center rows from the (replicated) centers table
with 4 indirect DMAs (one per 128-row chunk, pipelined against the
vector engine), computes sum((x-g)^2) per chunk via subtract +
fused square-reduce (scalar_tensor_tensor accum), and writes a [128,4]
partial-sum tile.  The host sums the partials in float64 and divides
by B*C.

Raw Bass (no Tile): the toolchain allows at most one semaphore wait
per compute instruction, so cross-engine deps are taken with
standalone wait_ge instructions instead of instruction-attached waits.
"""

import sys

if "/opt/trn_rl_repo" not in sys.path:
    sys.path.insert(0, "/opt/trn_rl_repo")

import numpy as np

import concourse.bass as bass
from concourse import mybir

NCORES = 8
B = 4096
D = 128
C = 20000
P = 128
BS = B // NCORES          # 512 rows per core
N = BS // P               # 4 rows per partition


def build_bass() -> bass.Bass:
    nc = bass.Bass(num_swdge_queues=2)
    x = nc.declare_dram_parameter("x", [BS, D], mybir.dt.float32, isOutput=False)
    idx = nc.declare_dram_parameter("idx", [BS], mybir.dt.int32, isOutput=False)
    centers = nc.declare_dram_parameter(
        "centers", [C, D], mybir.dt.float32, isOutput=False
    )
    out = nc.declare_dram_parameter("out", [P, N], mybir.dt.float32, isOutput=True)

    with (
        nc.sbuf_tensor([P, N], mybir.dt.int32) as idx_t,
        nc.sbuf_tensor([P, N, D], mybir.dt.float32) as x_t,
        nc.sbuf_tensor([P, N, D], mybir.dt.float32) as g_t,
        nc.sbuf_tensor([P, N, D], mybir.dt.float32) as d_t,
        nc.sbuf_tensor([P, N, D], mybir.dt.float32) as sq_t,
        nc.sbuf_tensor([P, N], mybir.dt.float32) as red_t,
        nc.semaphore("idx_sem") as idx_sem,
        nc.semaphore("x_sem") as x_sem,
        nc.semaphore("ga_sem") as ga_sem,
        nc.semaphore("gb_sem") as gb_sem,
        nc.semaphore("gc_sem") as gc_sem,
        nc.semaphore("gd_sem") as gd_sem,
        nc.semaphore("v_sem") as v_sem,
        nc.semaphore("done_sem") as done_sem,
    ):
        g_sems = [ga_sem, gb_sem, gc_sem, gd_sem]

        # Issue the input loads in `main`, before the Block bodies: they
        # start earlier and their completion overlaps the block entry
        # overhead.  (Kept: the Block-end barrier is load-bearing — it
        # keeps the NRT per-engine postamble from contending with
        # in-flight gather completion semaphores.)
        idx_dma = nc.sync.dma_start(
            out=idx_t[:], in_=idx[:].rearrange("(p n) -> p n", p=P)
        )
        # single_packet measured inert for 128-partition transfers
        # (walrus falls back above the per-packet descriptor limit);
        # kept because it is harmless and correctness-verified.
        idx_dma.ins.single_packet = True
        idx_dma.then_inc(idx_sem, 16)
        nc.sync.dma_start(
            out=x_t[:], in_=x[:].rearrange("(p n) d -> p n d", p=P)
        ).then_inc(x_sem, 16)

        with nc.Block(no_gpsimd_drain=True) as block:

            @block.sync
            def _(sync):
                sync.wait_ge(v_sem, 2 * N)
                # No wait on done_sem: the Sync queue drain at block end
                # guarantees the store lands before kernel completion.
                out_dma = sync.dma_start(out=out[:], in_=red_t[:])
                out_dma.ins.single_packet = True
                out_dma.then_inc(done_sem, 16)

            @block.gpsimd
            def _(gpsimd):
                gpsimd.wait_ge(idx_sem, 16)
                # HW honors only one offset per partition per indirect
                # DMA, so issue N gathers with [P, 1] offset tiles.
                for n in range(N):
                    gi = gpsimd.indirect_dma_start(
                        out=g_t[:, n, :],
                        out_offset=None,
                        in_=centers[:],
                        in_offset=bass.IndirectOffsetOnAxis(
                            ap=idx_t[:, n : n + 1], axis=0
                        ),
                    )
                    # alternate the two SWDGE queues so transfers overlap
                    if n % 2 == 1:
                        gi.ins.queue = "qPoolDynamic1"
                    gi.then_inc(g_sems[n], 16)

            @block.vector
            def _(vector):
                vector.wait_ge(x_sem, 16)
                # Chunk n computes while chunk n+1's gather is in
                # flight.  The v_sem chain between dependent DVE ops is
                # cheap (it overlaps the per-op pipeline DRAIN) and
                # keeps the race detector happy.
                for n in range(N):
                    vector.wait_ge(g_sems[n], 16)
                    vector.tensor_tensor(
                        out=d_t[:, n, :],
                        in0=x_t[:, n, :],
                        in1=g_t[:, n, :],
                        op=mybir.AluOpType.subtract,
                    ).then_inc(v_sem, 1)
                    vector.wait_ge(v_sem, 2 * n + 1)
                    # sq = (d + 0) * d ; accum = sum(sq) — fused
                    # square+reduce
                    vector.scalar_tensor_tensor(
                        out=sq_t[:, n, :],
                        in0=d_t[:, n, :],
                        scalar=0.0,
                        in1=d_t[:, n, :],
                        op0=mybir.AluOpType.add,
                        op1=mybir.AluOpType.mult,
                        accum_out=red_t[:, n : n + 1],
                    ).then_inc(v_sem, 1)

    if not nc.is_finalized():
        nc.finalize()
    return nc


_NC = None


def _get_nc() -> bass.Bass:
    global _NC
    if _NC is None:
        _NC = build_bass()
    return _NC


def make_in_maps(x, labels, centers):
    x = np.ascontiguousarray(np.asarray(x, dtype=np.float32))
    labels = np.asarray(labels).astype(np.int32)
    centers = np.ascontiguousarray(np.asarray(centers, dtype=np.float32))
    in_maps = []
    for c in range(NCORES):
        sl = slice(c * BS, (c + 1) * BS)
        in_maps.append(
            {
                "x": np.ascontiguousarray(x[sl]),
                "idx": np.ascontiguousarray(labels[sl]),
                "centers": centers,
            }
        )
    return in_maps


def reduce_outputs(results) -> np.ndarray:
    total = 0.0
    for r in results:
        total += float(np.sum(r["out"].astype(np.float64)))
    return np.array(np.float32(total / (B * C)))


def kernel(x, labels, centers) -> np.ndarray:
    from concourse.bass_utils import run_bass_kernel_spmd

    nc = _get_nc()
    in_maps = make_in_maps(x, labels, centers)
    res = run_bass_kernel_spmd(nc, in_maps, list(range(NCORES)))
    return reduce_outputs(res.results)

